# revision 17
# baseline (speedup 1.0000x reference)
"""Trainium2 Bass kernel for nn_Attention_30468497997979.

Reference computation (per batch b of 8):
    X = hidden_states[b,:,0,:]              # (C=768, S=384)
    Q/K/V = W @ X + b                       # 1x1 conv == channel matmul
    per head h (12 heads, head dim 64, channel c = d*12 + h):
        scores = (Q_h^T K_h) / 8, mask (keys k < q masked), softmax over k
        attn_h = V_h @ softmax
    out = Wo @ concat_heads(attn)           # channel c = h*64 + d
    Sharding: pure data-parallel, one batch per NeuronCore (8 cores).

Per-core kernel design (v4; v3 history in the git-less docstring of
kernel_v3_backup.py):
  - Host pre-permutes W_{q,k,v} rows to head-major channel order
    (c' = h*64 + d), transposes all weights to [c_in, c_out], and PACKS
    every input into its exact SBUF layout ([128 partitions, ...]), so
    each DMA moves one contiguous multi-KB run per partition.
    1/sqrt(d) folded into Wq/bq; V bias folded through attention into an
    output bias Wo @ bv (softmax rows sum to 1).  K bias DROPPED: it
    shifts every score in a softmax column by the same q.bk constant, so
    softmax (and the fused unnormalized-sum/denominator pair) is exactly
    invariant to it.  All matmul data bf16 (PSUM accumulation fp32).
  - HAM warmup: ~32 dummy matmuls on a zeroed tile run during the input
    DMA window, so the PE clock is at 8/8 (2.4 GHz) when real matmuls
    start (cold-start penalty is ~2us plus run-to-run variance).
  - Input DMAs: t1 (x + wq/wk chunk0) is the critical load; it is split
    in two halves on two queues (scalar + gpsimd) for full ~320 GB/s.
    The remaining weight loads are issued on the vector/sync queues with
    tc.tile_wait_until marks so the Tile scheduler cannot hoist them
    into the t1 window (rings on one queue drain CONCURRENTLY, and an
    engine with nothing else ready will otherwise pop them early).
  - scores are computed transposed ([k, q], keys on partitions) with
    causal trimming; the two heads of a chunk run as row-tile pairs
    (lhsT/rhs at partitions 0:64 vs 64:128 -> tile_position rows 0/64),
    interleaved instruction-by-instruction so the PE overlaps them and
    pulls LDWEIGHTS ahead (measured 16-67ns deltas vs 215ns serial).
  - softmax needs no max-subtraction (scores are O(1)).  The mask is
    applied multiplicatively AFTER exp on the GpSimd engine (no PSUM
    port needed: e-tiles are SBUF).
  - attn@V contracts over k on partitions (lhsT = per-head V^T tile
    with a fused ones-column computing the softmax denominator as PSUM
    row 64).  One 65-row copy per head lands rows+denominator in the
    chunk's AU tile (bf16).
  - Normalization is PER CHUNK (2 heads), fully overlapped with later
    chunks' attention: a K=65 matmul pair (lhsT = sel65, a 0/1 tile
    whose only nonzero row is 64) broadcasts the denominator row of AU
    straight from SBUF into a [128, S] PSUM tile -- no gather DMA, no
    repack DMA.  Then reciprocal_approx_fast (DVE, ~5x faster than
    reciprocal), a bf16 cast (scalar), and two [64, S] multiplies
    (DVE/GpSimd) produce the normalized chunk.
  - o_proj output chains open as attention chunks become ready (cc
    order 0..5 inside each chain), so the PE never idles while the last
    norm chains drain; chains use the freed scores-PSUM slots.
  - Schedule: chunks 0-2 project+score while wv loads; V-proj; chunks
    3-5 project+score interleaved with attention+normalization of
    chunks 0-2; attention 3-5; o_proj.  No PE gap is ever > ~2us, so
    the HAM clock never re-throttles mid-kernel.
"""

import numpy as np

B, C, S, H, D = 8, 768, 384, 12, 64
NC_CHUNKS = C // 128  # 6

_STATE = {}


# --------------------------------------------------------------------------
# Workaround: this walrus build rejects the multi-wait InstDrain that
# TileContext emits at exit ("Too many sync wait commands"). Split the
# drain's sem waits onto standalone sync-engine wait instructions.
def _patch_tile_drain():
    import concourse.tile as tile_mod
    from concourse.vector_clock import ScopedClock
    from bass_rust import SyncInfo

    if getattr(tile_mod.TileContext, "_drain_split_patch", False):
        return

    def _drain_and_barrier_split(self, tick_clock, wait_clock):
        nc = self.nc
        assert self.sems is not None
        handles = {}
        for h in self.sems.allocated().values():
            handles[h.num] = h
            handles[h.name] = h

        probe = nc.sync.nop()
        wait_clock.add_sem_waits(
            probe.ins, ScopedClock({None: tick_clock.global_clock})
        )
        waits = list(probe.ins.sync_info.on_wait)
        probe.ins.sync_info = SyncInfo(on_wait=[], on_update=[])
        for w in waits:
            h = handles.get(w.id) or handles.get(w.ant_name)
            if h is not None:
                nc.sync.wait_ge(h, w.wait_value)
            else:
                n2 = nc.sync.nop()
                n2.ins.sync_info = SyncInfo(on_wait=[w], on_update=[])

        drain_inst = nc.sync.drain()
        wait_clock.add_sem_waits(
            drain_inst.ins, ScopedClock({None: tick_clock.global_clock})
        )
        if list(drain_inst.ins.sync_info.on_wait):
            drain_inst.ins.sync_info = SyncInfo(on_wait=[], on_update=[])

        nc.all_engine_barrier()
        popped = nc._tile_sem_poison_stack.pop()
        assert popped is self._sem_poison
        nc.clear_and_free_semaphores(list(self.sems.allocated().values()))
        nc.all_engine_barrier()

        # This walrus codegen supports at most ONE sem wait per
        # instruction. Move extra waits onto same-engine nop carriers
        # inserted just before the instruction (engine queues execute in
        # order, so the semantics are identical).
        import concourse.mybir as mybir

        k = 0
        for f in nc.m.functions:
            for bb in f.blocks:
                new_insts = []
                for inst in bb.instructions:
                    si = inst.sync_info
                    waits = list(si.on_wait) if si else []
                    if len(waits) > 1:
                        for w in waits[:-1]:
                            nop = mybir.InstNoOp(name=f"I-wsplit-{k}")
                            k += 1
                            nop.engine = inst.engine
                            nop.sync_info = SyncInfo(on_wait=[w], on_update=[])
                            nc.register_instruction(nop)
                            new_insts.append(nop)
                        inst.sync_info = SyncInfo(
                            on_wait=[waits[-1]], on_update=list(si.on_update)
                        )
                    new_insts.append(inst)
                bb.instructions = new_insts

    tile_mod.TileContext._drain_and_barrier = _drain_and_barrier_split
    tile_mod.TileContext._drain_split_patch = True


# --------------------------------------------------------------------------
def _build_nc():
    import concourse.bass as bass
    import concourse.mybir as mybir
    import concourse.tile as tile

    _patch_tile_drain()

    f32 = mybir.dt.float32
    bf16 = mybir.dt.bfloat16
    Ident = mybir.ActivationFunctionType.Identity
    Copy = mybir.ActivationFunctionType.Copy
    Exp = mybir.ActivationFunctionType.Exp

    nc = bass.Bass()
    # t1 halves: [x | wq0 | wk0 | wq1 | wk1] for in-chunks 0-2 / 3-5.
    t1a_d = nc.dram_tensor("t1a", [128, 2688], bf16, kind="ExternalInput")
    t1b_d = nc.dram_tensor("t1b", [128, 2688], bf16, kind="ExternalInput")
    # wq/wk for output chunks 2..5, grouped for staged release
    t21_d = nc.dram_tensor("t21", [128, 1, 2, NC_CHUNKS, 128], bf16, kind="ExternalInput")
    t22_d = nc.dram_tensor("t22", [128, 1, 2, NC_CHUNKS, 128], bf16, kind="ExternalInput")
    t2b_d = nc.dram_tensor("t2b", [128, 2, 2, NC_CHUNKS, 128], bf16, kind="ExternalInput")
    wv_d = nc.dram_tensor("wvt", [128, 2, NC_CHUNKS, 384], bf16, kind="ExternalInput")
    wo_d = nc.dram_tensor("wot", [128, NC_CHUNKS, C], bf16, kind="ExternalInput")
    # packed constants [128, 204] f32:
    #   cols 0:6 bq (col=chunk), 6:12 obias (= Wo @ bv'),
    #   cols 12:140 = [128, 256] bf16 = 0/1 lower-triangle (k>=q) twice,
    #   rows 0:2 of cols 140:204 = [2, 128] bf16 = sel2 broadcast mask
    cst_d = nc.dram_tensor("cst", [128, 204], f32, kind="ExternalInput")
    y_d = nc.dram_tensor("y", [128, NC_CHUNKS, S], bf16, kind="ExternalOutput")
    # scratch targets for the probe DMAs that gate staged weight loads
    scr = [
        nc.dram_tensor(f"scr{i}", [1, 16], bf16, kind="Internal")
        for i in range(5)
    ]

    with tile.TileContext(nc) as tc:
        with (
            tc.tile_pool(name="persist", bufs=1) as persist,
            tc.tile_pool(name="epool", bufs=9) as epool,
            tc.tile_pool(name="npool", bufs=2) as npool,
            tc.tile_pool(name="psA", bufs=2, space="PSUM") as psA,
            tc.tile_pool(name="psS", bufs=4, space="PSUM") as psS,
            tc.tile_pool(name="psV", bufs=2, space="PSUM") as psV,
        ):
            # ---- input loads -----------------------------------------
            t1 = persist.tile([128, 2, 2688], bf16, tag="t1", name="t1")
            t2 = persist.tile(
                [128, 4, 2, NC_CHUNKS, 128], bf16, tag="t2", name="t2"
            )
            wv_sb = persist.tile([128, 2, NC_CHUNKS, 384], bf16, tag="wv", name="wv")
            wo_sb = persist.tile([128, NC_CHUNKS, C], bf16, tag="wo", name="wo")
            cst = persist.tile([128, 204], f32, tag="cst", name="cst")

            # Critical first load at full bandwidth on two queues.  Later
            # loads are held back by tiny "probe" DMAs whose data deps
            # stall the issuing engine until earlier data has landed
            # (rings on one queue drain CONCURRENTLY, so an ungated later
            # dma_start steals bandwidth from the critical t1 load; and
            # the Tile scheduler hoists any ready instruction when an
            # engine has nothing else to do).
            nc.scalar.dma_start(t1[:, 0], t1a_d[:, :])
            nc.gpsimd.dma_start(t1[:, 1], t1b_d[:, :])
            nc.sync.dma_start(cst[:], cst_d[:, :])

            # ---- on-chip constants and warmup ------------------------
            # wu: zeroed tile for HAM warmup matmuls. sel2 (a view of
            # cst): 0/1 selector for the K=2 denominator-reciprocal
            # broadcast (row p lights up partition half p).
            wu = persist.tile([128, 128], bf16, tag="wu", name="wu")
            nc.gpsimd.memset(wu[:], 0.0)
            sel2 = cst[0:2, 140:204].bitcast(bf16)
            vt = []
            for sq in range(3):
                t = persist.tile([128, H, D + 1], bf16, tag=f"vt{sq}", name=f"vt{sq}")
                nc.gpsimd.memset(t[:, :, D : D + 1], 1.0)
                vt.append(t)

            # ~32 dummy matmuls keep the PE busy through the load window
            # so HAM is at K=8/8 when real matmuls start.
            ps_wu = psS.tile([128, S], f32, tag="sc", name="ps_wu")
            for i in range(32):
                nc.tensor.matmul(
                    ps_wu[:, 0:128], wu[:], wu[:],
                    start=True, stop=True, skip_group_check=True,
                )

            # ---- staged weight loads ---------------------------------
            # oc2 weights on the sync queue once the (tiny) cst load is
            # done -- overlaps only the t1 tail
            nc.sync.dma_start(scr[4][:, :], cst[0:1, 0:8].bitcast(bf16))
            nc.sync.dma_start(t2[:, 0:1], t21_d[:, :, :, :, :])
            # wv on the scalar queue once t1a is done
            nc.scalar.dma_start(scr[0][:, :], t1[0:1, 0, 0:16])
            nc.scalar.dma_start(wv_sb[:], wv_d[:, :, :, :])
            # oc4-5 weights on the gpsimd queue once t1b is done
            nc.gpsimd.dma_start(scr[1][:, :], t1[0:1, 1, 0:16])
            nc.gpsimd.dma_start(t2[:, 2:4], t2b_d[:, :, :, :, :])
            # oc3 weights once t1 is done; wo later (see _late_loads)
            nc.sync.dma_start(scr[2][:, :], t1[0:1, 1, 16:32])
            nc.sync.dma_start(t2[:, 1:2], t22_d[:, :, :, :, :])

            def _late_loads():
                nc.sync.dma_start(scr[3][:, :], k_sb[1][0:1, 0:16])
                nc.sync.dma_start(wo_sb[:], wo_d[:, :, :])

            def xt(cc):
                return t1[:, cc // 3, (cc % 3) * 384 : (cc % 3) * 384 + 384]

            def wslice(w, oc, cc):
                # w: 0 = wq, 1 = wk; chunks 0-1 live in t1, rest in t2
                if oc < 2:
                    base = 1152 + (oc * 2 + w) * 384 + (cc % 3) * 128
                    return t1[:, cc // 3, base : base + 128]
                return t2[:, oc - 2, w, cc, :]

            # [128, 2, 128] view of the doubled 0/1 triangle
            tri2 = cst[:, 12:140].bitcast(bf16).rearrange("p (a q) -> p a q", q=128)

            # ---- persistent working tiles ----------------------------
            q_sb = [
                persist.tile([128, S], bf16, tag=f"q{oc}", name=f"q{oc}")
                for oc in range(NC_CHUNKS)
            ]
            k_sb = [
                persist.tile([128, S], bf16, tag=f"k{oc}", name=f"k{oc}")
                for oc in range(NC_CHUNKS)
            ]
            attn_sb = [
                persist.tile([128, S], bf16, tag=f"at{oc}", name=f"at{oc}")
                for oc in range(NC_CHUNKS)
            ]
            # per-chunk unnormalized attn rows 0:64 + denominator row 64
            AU = [
                persist.tile([D + 1, 2, S], f32, tag=f"au{c}", name=f"au{c}")
                for c in range(NC_CHUNKS)
            ]
            ot = persist.tile([128, NC_CHUNKS, S], bf16, tag="ot", name="ot")

            # ---- stage helpers ---------------------------------------
            def qkproj(oc):
                # Q then K chain; K bias dropped (softmax-invariant)
                ps_q = psA.tile([128, S], f32, tag="proj", name="ps_q")
                for cc in range(NC_CHUNKS):
                    nc.tensor.matmul(
                        ps_q[:], wslice(0, oc, cc), xt(cc),
                        start=(cc == 0), stop=(cc == NC_CHUNKS - 1),
                    )
                nc.vector.tensor_scalar_add(q_sb[oc][:], ps_q[:], cst[:, oc : oc + 1])
                ps_k = psA.tile([128, S], f32, tag="proj", name="ps_k")
                for cc in range(NC_CHUNKS):
                    nc.tensor.matmul(
                        ps_k[:], wslice(1, oc, cc), xt(cc),
                        start=(cc == 0), stop=(cc == NC_CHUNKS - 1),
                    )
                nc.vector.tensor_copy(k_sb[oc][:], ps_k[:])

            def scores_pair(oc):
                # Both heads of the chunk as row-tile pairs (partitions
                # 0:64 / 64:128 -> PE row groups 0-1 / 2-3): interleaved
                # emission lets the PE run them concurrently and pull
                # LDWEIGHTS ahead.  Causal trimming: psa = [kc0 q0:128 |
                # kc1 q0:256], psb = kc2 q0:384.  exp straight from
                # PSUM; 0/1 triangle applied after on the diagonal
                # sub-blocks (gpsimd, SBUF).
                Qh = [q_sb[oc][0:D, :], q_sb[oc][D : 2 * D, :]]
                Kh = [k_sb[oc][0:D, :], k_sb[oc][D : 2 * D, :]]
                psa = [
                    psS.tile([128, S], f32, tag="sc", name=f"psa{p}")
                    for p in range(2)
                ]
                for p in range(2):
                    nc.tensor.matmul(
                        psa[p][:, 0:128], Kh[p][:, 0:128], Qh[p][:, 0:128],
                        start=True, stop=True, skip_group_check=True,
                    )
                for p in range(2):
                    nc.tensor.matmul(
                        psa[p][:, 128:384], Kh[p][:, 128:256], Qh[p][:, 0:256],
                        start=True, stop=True, skip_group_check=True,
                    )
                psb = [
                    psS.tile([128, S], f32, tag="sc", name=f"psb{p}")
                    for p in range(2)
                ]
                for p in range(2):
                    nc.tensor.matmul(
                        psb[p][:], Kh[p][:, 256:384], Qh[p][:, 0:384],
                        start=True, stop=True, skip_group_check=True,
                    )
                out = []
                for p in range(2):
                    eA = epool.tile([128, 512], bf16, tag="eA", name="eA")
                    nc.scalar.activation(eA[:, 0:S], psa[p][:], Exp)
                    eB = epool.tile([128, S], bf16, tag="eB", name="eB")
                    nc.scalar.activation(eB[:], psb[p][:], Exp)
                    # eA is 512 wide so its two diagonal sub-blocks (cols
                    # 0:128 and 256:384) form one uniform-stride
                    # [128,2,128] AP for a single masked multiply.
                    diag2 = eA[:].rearrange("p (a q) -> p a q", q=256)[:, :, 0:128]
                    nc.gpsimd.tensor_mul(diag2, diag2, tri2)
                    nc.gpsimd.tensor_mul(eB[:, 256:384], eB[:, 256:384], tri2[:, 0, :])
                    out.append((eA, eB))
                return out

            def vproj():
                # lhsT = x block (reused for both halves), rhs = wv.
                # half 0 = heads 0-5 first so chunk-0 attention can start
                # after the first three chains.
                for half in range(2):
                    for sq in range(3):
                        ps_v = psA.tile([128, S], f32, tag="proj", name="ps_v")
                        for cc in range(NC_CHUNKS):
                            nc.tensor.matmul(
                                ps_v[:],
                                xt(cc)[:, sq * 128 : (sq + 1) * 128],
                                wv_sb[:, half, cc, :],
                                start=(cc == 0), stop=(cc == NC_CHUNKS - 1),
                            )
                        dst = vt[sq][:, half * 6 : (half + 1) * 6, 0:D]
                        src = ps_v[:].rearrange("p (h d) -> p h d", d=D)
                        if half == 0:
                            nc.vector.tensor_copy(dst, src)
                        else:
                            nc.scalar.activation(dst, src, Copy)

            def av(h, eA, eB):
                # accumulate widest first so every element's first write
                # carries the start flag
                ps_av = psV.tile([D + 1, S], f32, tag="av", name="ps_av")
                nc.tensor.matmul(
                    ps_av[:, 0:384], vt[2][:, h, :], eB[:, 0:384],
                    start=True, stop=False, skip_group_check=True,
                )
                nc.tensor.matmul(
                    ps_av[:, 0:256], vt[1][:, h, :], eA[:, 128:384],
                    start=False, stop=False, skip_group_check=True,
                )
                nc.tensor.matmul(
                    ps_av[:, 0:128], vt[0][:, h, :], eA[:, 0:128],
                    start=False, stop=True, skip_group_check=True,
                )
                dst = AU[h // 2][0 : D + 1, h % 2, :]
                if h % 2 == 0:
                    nc.vector.tensor_copy(dst, ps_av[:, :])
                else:
                    nc.scalar.activation(dst, ps_av[:, :], Copy)

            def norm(c):
                # Denominators -> [12, 64] rows (reciprocal cost is
                # free-dim bound, so spread over partitions), reciprocal
                # (DVE), bf16 cast (scalar Copy -- same ACT table as
                # Exp, no table switch), repack to [2, 384], then a K=2
                # bf16 matmul broadcasts each head's 1/sum row across
                # its partition half; two multiplies (f32 SBUF x f32
                # PSUM -- mismatched base partitions are legal when one
                # operand is PSUM) normalize the chunk.
                s12 = npool.tile([12, 64], f32, tag="s12", name="s12")
                nc.sync.dma_start(s12[:], AU[c][D : D + 1, :, :])
                r12 = npool.tile([12, 64], f32, tag="r12", name="r12")
                nc.vector.reciprocal(r12[:], s12[:])
                rb12 = npool.tile([12, 64], bf16, tag="rb12", name="rb12")
                nc.scalar.activation(rb12[:], r12[:], Copy)
                rr = npool.tile([2, S], bf16, tag="rr", name="rr")
                nc.gpsimd.dma_start(
                    rr[:].rearrange("p (b q) -> p b q", q=64), rb12[:]
                )
                ps_n = psS.tile([128, S], f32, tag="sc", name="ps_n")
                nc.tensor.matmul(
                    ps_n[:], sel2[:], rr[:],
                    start=True, stop=True, skip_group_check=True,
                )
                for par in range(2):
                    nc.vector.tensor_mul(
                        attn_sb[c][par * D : (par + 1) * D, :],
                        AU[c][0:D, par, :],
                        ps_n[par * D : (par + 1) * D, :],
                    )

            o_ps = {}

            def oproj(oc, ccs, start, stop, pool):
                if oc in o_ps:
                    ps = o_ps[oc]
                else:
                    ps = o_ps[oc] = pool.tile(
                        [128, S], f32, tag=pool is psA and "proj" or "sc", name="ps_o"
                    )
                for i, cc in enumerate(ccs):
                    nc.tensor.matmul(
                        ps[:],
                        wo_sb[:, cc, oc * 128 : (oc + 1) * 128],
                        attn_sb[cc],
                        start=(start and i == 0),
                        stop=(stop and i == len(ccs) - 1),
                        skip_group_check=True,
                    )
                if stop:
                    del o_ps[oc]
                    nc.scalar.activation(
                        ot[:, oc, :], ps[:], Ident, bias=cst[:, 6 + oc : 7 + oc]
                    )
                    if oc % 2 == 1:
                        nc.sync.dma_start(
                            y_d[:, oc - 1 : oc + 1, :], ot[:, oc - 1 : oc + 1, :]
                        )

            # ---- schedule --------------------------------------------
            e_tiles = {}
            for oc in (0, 1, 2, 3):
                qkproj(oc)
                if oc == 1:
                    _late_loads()
                pair = scores_pair(oc)
                e_tiles[2 * oc] = pair[0]
                e_tiles[2 * oc + 1] = pair[1]
            vproj()
            for oc in (4, 5):
                c = oc - 4
                av(2 * c, *e_tiles.pop(2 * c))
                av(2 * c + 1, *e_tiles.pop(2 * c + 1))
                norm(c)
                qkproj(oc)
                pair = scores_pair(oc)
                e_tiles[2 * oc] = pair[0]
                e_tiles[2 * oc + 1] = pair[1]
            for c in (2, 3, 4, 5):
                av(2 * c, *e_tiles.pop(2 * c))
                av(2 * c + 1, *e_tiles.pop(2 * c + 1))
                norm(c)
            # o_proj: chains open on the attn prefix (cc 0..4) and close
            # with cc5; chains 0-3 use the freed scores slots, 4-5 psA.
            for oc in (0, 1, 2, 3):
                oproj(oc, (0, 1, 2, 3, 4), start=True, stop=False, pool=psS)
            for oc in (4, 5):
                oproj(oc, (0, 1, 2, 3, 4), start=True, stop=False, pool=psA)
            for oc in range(6):
                oproj(oc, (5,), start=False, stop=True, pool=None)

    return nc


def _get_nc():
    if "nc" not in _STATE:
        _STATE["nc"] = _build_nc()
    return _STATE["nc"]


# --------------------------------------------------------------------------
def _prep_maps(inputs):
    import ml_dtypes

    bf16 = ml_dtypes.bfloat16

    hs = np.asarray(inputs["hidden_states"], dtype=np.float32)
    Wq = np.asarray(inputs["Wq"], dtype=np.float32)
    bq = np.asarray(inputs["bq"], dtype=np.float32)
    Wk = np.asarray(inputs["Wk"], dtype=np.float32)
    Wv = np.asarray(inputs["Wv"], dtype=np.float32)
    bv = np.asarray(inputs["bv"], dtype=np.float32)
    Wo = np.asarray(inputs["Wo"], dtype=np.float32)

    # head-major channel permutation: c' = h*64 + d  <-  c = d*12 + h
    idx = (np.arange(H)[:, None] + np.arange(D)[None, :] * H).reshape(C)
    scale = float(D) ** -0.5

    wqt = np.ascontiguousarray((scale * Wq[idx, :]).T).astype(bf16)
    wkt = np.ascontiguousarray(Wk[idx, :].T).astype(bf16)
    wvt = np.ascontiguousarray(Wv[idx, :].T).astype(bf16)
    wot = np.ascontiguousarray(Wo.T).astype(bf16)

    # packed constants [128, 204] f32
    cstf = np.zeros((128, 204), dtype=np.float32)
    cstf[:, 0:6] = (scale * bq[idx]).reshape(NC_CHUNKS, 128).T
    # V-bias folded through attention (softmax rows sum to 1)
    cstf[:, 6:12] = (Wo @ bv[idx]).reshape(NC_CHUNKS, 128).T
    # 0/1 triangle: allowed keys are k >= q -> tri[k, q] = 1 iff k >= q
    tri = np.tril(np.ones((128, 128), dtype=np.float32)).astype(bf16)
    cstf[:, 12:140] = np.tile(tri, (1, 2)).view(np.float32)
    sel = np.zeros((2, 128), dtype=np.float32)
    sel[0, 0:64] = 1.0
    sel[1, 64:128] = 1.0
    cstf[0:2, 140:204] = sel.astype(bf16).view(np.float32)

    # pack [c_in, c_out] weights into their SBUF layouts (see _build_nc)
    nch = NC_CHUNKS
    wqp = np.ascontiguousarray(
        wqt.reshape(nch, 128, nch, 128).transpose(1, 2, 0, 3)
    )  # [p, out_chunk, in_chunk, col]
    wkp = np.ascontiguousarray(wkt.reshape(nch, 128, nch, 128).transpose(1, 2, 0, 3))
    wvp = np.ascontiguousarray(
        wvt.reshape(nch, 128, 2, 384).transpose(1, 2, 0, 3)
    )  # [p, half, in_chunk, col]
    wop = np.ascontiguousarray(wot.reshape(nch, 128, C).transpose(1, 0, 2))

    t2 = np.stack([wqp[:, 2:6], wkp[:, 2:6]], axis=2)  # [128, 4, 2, 6, 128]
    shared = {
        "t21": np.ascontiguousarray(t2[:, 0:1]),
        "t22": np.ascontiguousarray(t2[:, 1:2]),
        "t2b": np.ascontiguousarray(t2[:, 2:4]),
        "wvt": wvp,
        "wot": wop,
        "cst": cstf,
    }
    maps = []
    for b in range(B):
        xb = hs[b, :, 0, :].astype(bf16)
        xp = xb.reshape(nch, 128, S).transpose(1, 0, 2)  # [p, cc, S]
        halves = {}
        for h, name in ((0, "t1a"), (1, "t1b")):
            ccs = slice(3 * h, 3 * h + 3)
            halves[name] = np.ascontiguousarray(
                np.concatenate(
                    [
                        xp[:, ccs].reshape(128, 1152),
                        wqp[:, 0, ccs].reshape(128, 384),
                        wkp[:, 0, ccs].reshape(128, 384),
                        wqp[:, 1, ccs].reshape(128, 384),
                        wkp[:, 1, ccs].reshape(128, 384),
                    ],
                    axis=1,
                )
            )
        maps.append({**halves, **shared})
    return maps


def _run(inputs, trace=False, **kwargs):
    from concourse.bass_utils import run_bass_kernel_spmd

    nc = _get_nc()
    in_maps = _prep_maps(inputs)
    res = run_bass_kernel_spmd(
        nc, in_maps, core_ids=list(range(B)), trace=trace, **kwargs
    )
    out = np.stack(
        [
            np.asarray(res.results[b]["y"])
            .astype(np.float32)
            .transpose(1, 0, 2)  # [p, cc, s] -> [cc, p, s]
            .reshape(C, S)
            for b in range(B)
        ],
        axis=0,
    )
    return out.reshape(B, C, 1, S), res


def kernel(**inputs):
    out, _ = _run(inputs, trace=False)
    return out


# revision 19
# speedup vs baseline: 1.1098x; 1.1098x over previous
"""Trainium2 Bass kernel for nn_Attention_30468497997979.

Reference computation (per batch b of 8):
    X = hidden_states[b,:,0,:]              # (C=768, S=384)
    Q/K/V = W @ X + b                       # 1x1 conv == channel matmul
    per head h (12 heads, head dim 64, channel c = d*12 + h):
        scores = (Q_h^T K_h) / 8, mask (keys k < q masked), softmax over k
        attn_h = V_h @ softmax
    out = Wo @ concat_heads(attn)           # channel c = h*64 + d
    Sharding: pure data-parallel, one batch per NeuronCore (8 cores).

Per-core kernel design (v5; v3 in kernel_v3_backup.py):
  - Host pre-permutes W_{q,k,v} rows to head-major channel order
    (c' = h*64 + d), transposes all weights to [c_in, c_out], and PACKS
    every input into its exact SBUF layout ([128 partitions, ...]), so
    each DMA moves one contiguous multi-KB run per partition.
    1/sqrt(d) folded into Wq/bq; V bias folded through attention into an
    output bias Wo @ bv (softmax rows sum to 1).  K bias DROPPED: it
    shifts every score in a softmax column by the same q.bk constant, so
    softmax (and the fused unnormalized-sum/denominator pair) is exactly
    invariant to it.  All matmul data bf16 (PSUM accumulation fp32).
  - DMA queues are NOT symmetric: only the scalar engine's queue
    sustains ~300+ GB/s; the gpsimd and sync queues crawl at 50-90 GB/s
    under contention (measured).  So ALL deadline-critical loads go on
    the scalar queue, phased so that concurrent rings (which drain
    round-robin, i.e. finish in SIZE order) complete before their
    consumer: phase 1 = x+wq0/wk0 | oc1 | oc2, phase 2 (released by a
    probe DMA whose data dep stalls the scalar engine until t1 landed) =
    oc3 | oc4-5, phase 3 = wv.  wo rides the slow gpsimd queue from the
    start; cst rides sync.  The probes also stop the Tile scheduler from
    hoisting later loads into the t1 window (an engine with nothing else
    ready pops any ready instruction regardless of priority).
  - HAM warmup: ~40 dummy matmuls on a zeroed tile run during the input
    DMA window, so the PE clock is at 8/8 (2.4 GHz) when real matmuls
    start (the cold-start penalty is ~2us plus run-to-run variance).
  - scores are computed transposed ([k, q], keys on partitions) with
    causal trimming; the two heads of a chunk run as row-tile pairs
    (operands at partitions 0:64 vs 64:128 -> PE row groups 0-1/2-3),
    interleaved instruction-by-instruction so the PE overlaps them and
    pulls LDWEIGHTS ahead (measured 16-67ns issue deltas vs 215 serial).
  - softmax needs no max-subtraction (scores are O(1)).  The mask is
    applied multiplicatively AFTER exp: exp(s+m) = exp(s)*{0,1}, on the
    GpSimd engine (e-tiles are SBUF; GpSimd has no PSUM port).
  - attn@V contracts over k on partitions (lhsT = per-head V^T tile
    with a fused ones-column computing the softmax denominator as PSUM
    row 64).  One 65-row copy per head lands rows+denominator in the
    chunk's AU tile (f32).
  - Normalization is PER CHUNK (2 heads), overlapped with later chunks'
    attention: denominators gather to [12, 64] rows (reciprocal cost is
    free-dim bound, so spread over partitions; sync queue), DVE
    reciprocal, bf16 cast (scalar Copy -- same ACT table as Exp, no
    table-switch), repack to [2, 384] (gpsimd queue), then a K=2 bf16
    matmul broadcasts each head's 1/sum row across its partition half;
    two multiplies (f32 SBUF x f32 PSUM -- mismatched base partitions
    are legal when one operand is PSUM) normalize the chunk.
  - o_proj chains open on the attn prefix (cc 0..4) as chunks become
    ready and close with cc5.  Six concurrent accumulators: 2 from psA
    (free after qkproj(5)), 2 from psV (free after the last attention
    copy), 2 from psS -- allocated AFTER all ps_n tiles so the pool
    rings never make an o_proj chain wait on a late normalization.
  - Engine split: Scalar = exps + dma issue + half the copies + casts +
    output bias; DVE = q bias-add + k copies + reciprocals + normalize
    multiplies + half the copies; GpSimd = masks + wo issue + repacks;
    Sync = cst + gathers + stores; PE bound overall.
"""

import numpy as np

B, C, S, H, D = 8, 768, 384, 12, 64
NC_CHUNKS = C // 128  # 6

_STATE = {}


# --------------------------------------------------------------------------
# Workaround: this walrus build rejects the multi-wait InstDrain that
# TileContext emits at exit ("Too many sync wait commands"). Split the
# drain's sem waits onto standalone sync-engine wait instructions.
def _patch_tile_drain():
    import concourse.tile as tile_mod
    from concourse.vector_clock import ScopedClock
    from bass_rust import SyncInfo

    if getattr(tile_mod.TileContext, "_drain_split_patch", False):
        return

    def _drain_and_barrier_split(self, tick_clock, wait_clock):
        nc = self.nc
        assert self.sems is not None
        handles = {}
        for h in self.sems.allocated().values():
            handles[h.num] = h
            handles[h.name] = h

        probe = nc.sync.nop()
        wait_clock.add_sem_waits(
            probe.ins, ScopedClock({None: tick_clock.global_clock})
        )
        waits = list(probe.ins.sync_info.on_wait)
        probe.ins.sync_info = SyncInfo(on_wait=[], on_update=[])
        for w in waits:
            h = handles.get(w.id) or handles.get(w.ant_name)
            if h is not None:
                nc.sync.wait_ge(h, w.wait_value)
            else:
                n2 = nc.sync.nop()
                n2.ins.sync_info = SyncInfo(on_wait=[w], on_update=[])

        drain_inst = nc.sync.drain()
        wait_clock.add_sem_waits(
            drain_inst.ins, ScopedClock({None: tick_clock.global_clock})
        )
        if list(drain_inst.ins.sync_info.on_wait):
            drain_inst.ins.sync_info = SyncInfo(on_wait=[], on_update=[])

        nc.all_engine_barrier()
        popped = nc._tile_sem_poison_stack.pop()
        assert popped is self._sem_poison
        nc.clear_and_free_semaphores(list(self.sems.allocated().values()))
        nc.all_engine_barrier()

        # This walrus codegen supports at most ONE sem wait per
        # instruction. Move extra waits onto same-engine nop carriers
        # inserted just before the instruction (engine queues execute in
        # order, so the semantics are identical).
        import concourse.mybir as mybir

        k = 0
        for f in nc.m.functions:
            for bb in f.blocks:
                new_insts = []
                for inst in bb.instructions:
                    si = inst.sync_info
                    waits = list(si.on_wait) if si else []
                    if len(waits) > 1:
                        for w in waits[:-1]:
                            nop = mybir.InstNoOp(name=f"I-wsplit-{k}")
                            k += 1
                            nop.engine = inst.engine
                            nop.sync_info = SyncInfo(on_wait=[w], on_update=[])
                            nc.register_instruction(nop)
                            new_insts.append(nop)
                        inst.sync_info = SyncInfo(
                            on_wait=[waits[-1]], on_update=list(si.on_update)
                        )
                    new_insts.append(inst)
                bb.instructions = new_insts

    tile_mod.TileContext._drain_and_barrier = _drain_and_barrier_split
    tile_mod.TileContext._drain_split_patch = True


# --------------------------------------------------------------------------
def _build_nc():
    import concourse.bass as bass
    import concourse.mybir as mybir
    import concourse.tile as tile

    _patch_tile_drain()

    f32 = mybir.dt.float32
    bf16 = mybir.dt.bfloat16
    Ident = mybir.ActivationFunctionType.Identity
    Copy = mybir.ActivationFunctionType.Copy
    Exp = mybir.ActivationFunctionType.Exp

    nc = bass.Bass()
    # t1 = [x (6 cc) | wq0 | wk0], the critical first load
    t1_d = nc.dram_tensor("t1", [128, 3840], bf16, kind="ExternalInput")
    # wq/wk for output chunks 1..5 (phased: oc1, oc2, oc3, oc4-5)
    t2_d = [
        nc.dram_tensor(
            f"t2{i}",
            [128, 2 if i == 3 else 1, 2, NC_CHUNKS, 128],
            bf16,
            kind="ExternalInput",
        )
        for i in range(4)
    ]
    wv_d = nc.dram_tensor("wvt", [128, 2, NC_CHUNKS, 384], bf16, kind="ExternalInput")
    wo_d = nc.dram_tensor("wot", [128, NC_CHUNKS, C], bf16, kind="ExternalInput")
    # packed constants [128, 204] f32:
    #   cols 0:6 bq (col=chunk), 6:12 obias (= Wo @ bv'),
    #   cols 12:140 = [128, 256] bf16 = 0/1 lower-triangle (k>=q) twice,
    #   rows 0:2 of cols 140:204 = [2, 128] bf16 = sel2 broadcast mask
    cst_d = nc.dram_tensor("cst", [128, 204], f32, kind="ExternalInput")
    y_d = nc.dram_tensor("y", [128, NC_CHUNKS, S], bf16, kind="ExternalOutput")
    # scratch targets for the probe DMAs that gate staged weight loads
    scr = [
        nc.dram_tensor(f"scr{i}", [1, 16], bf16, kind="Internal")
        for i in range(3)
    ]

    with tile.TileContext(nc) as tc:
        with (
            tc.tile_pool(name="persist", bufs=1) as persist,
            tc.tile_pool(name="epool", bufs=12) as epool,
            tc.tile_pool(name="npool", bufs=2) as npool,
            tc.tile_pool(name="psA", bufs=2, space="PSUM") as psA,
            tc.tile_pool(name="psS", bufs=4, space="PSUM") as psS,
            tc.tile_pool(name="psV", bufs=2, space="PSUM") as psV,
        ):
            # ---- input loads: phase 1 --------------------------------
            t1 = persist.tile([128, 3840], bf16, tag="t1", name="t1")
            t2 = persist.tile(
                [128, 5, 2, NC_CHUNKS, 128], bf16, tag="t2", name="t2"
            )
            wv_sb = persist.tile([128, 2, NC_CHUNKS, 384], bf16, tag="wv", name="wv")
            wo_sb = persist.tile([128, NC_CHUNKS, C], bf16, tag="wo", name="wo")
            cst = persist.tile([128, 204], f32, tag="cst", name="cst")

            # Concurrent rings on one queue drain round-robin, so they
            # finish in size order: oc1/oc2 (small) land before t1.
            nc.scalar.dma_start(t1[:], t1_d[:, :])
            nc.scalar.dma_start(t2[:, 0:1], t2_d[0][:, :, :, :, :])
            nc.scalar.dma_start(t2[:, 1:2], t2_d[1][:, :, :, :, :])
            nc.sync.dma_start(cst[:], cst_d[:, :])
            # wo rides the slow gpsimd queue in the background; gated on
            # cst so its descriptors don't race the t1 window
            nc.gpsimd.dma_start(scr[0][:, :], cst[0:1, 0:8].bitcast(bf16))
            nc.gpsimd.dma_start(wo_sb[:], wo_d[:, :, :])

            def _phase2_loads():
                # released once t1 has fully landed (probe stalls the
                # scalar engine, which is otherwise idle here)
                nc.scalar.dma_start(scr[1][:, :], t1[0:1, 0:16])
                nc.scalar.dma_start(t2[:, 2:3], t2_d[2][:, :, :, :, :])
                nc.scalar.dma_start(t2[:, 3:5], t2_d[3][:, :, :, :, :])

            def _phase3_loads():
                nc.scalar.dma_start(wv_sb[:], wv_d[:, :, :, :])

            def xt(cc):
                return t1[:, cc * 384 : (cc + 1) * 384]

            def wslice(w, oc, cc):
                # w: 0 = wq, 1 = wk; chunk 0 lives in t1, the rest in t2
                if oc == 0:
                    base = 2304 + w * 768 + cc * 128
                    return t1[:, base : base + 128]
                return t2[:, oc - 1, w, cc, :]

            # ---- on-chip constants and warmup ------------------------
            # wu: zeroed tile for HAM warmup matmuls. sel2 (a view of
            # cst): 0/1 selector for the K=2 denominator-reciprocal
            # broadcast (row p lights up partition half p).
            wu = persist.tile([128, 192], bf16, tag="wu", name="wu")
            nc.gpsimd.memset(wu[:], 0.0)
            sel2 = cst[0:2, 140:204].bitcast(bf16)
            vt = []
            for sq in range(3):
                t = persist.tile([128, H, D + 1], bf16, tag=f"vt{sq}", name=f"vt{sq}")
                nc.gpsimd.memset(t[:, :, D : D + 1], 1.0)
                vt.append(t)

            # dummy matmuls keep the PE busy through the load window so
            # HAM is at K=8/8 when real matmuls start
            ps_wu = psS.tile([128, S], f32, tag="sc", name="ps_wu")
            for i in range(40):
                nc.tensor.matmul(
                    ps_wu[:, 0:192], wu[:, 0:128], wu[:],
                    start=True, stop=True, skip_group_check=True,
                )

            # [128, 2, 128] view of the doubled 0/1 triangle
            tri2 = cst[:, 12:140].bitcast(bf16).rearrange("p (a q) -> p a q", q=128)

            # ---- persistent working tiles ----------------------------
            q_sb = [
                persist.tile([128, S], bf16, tag=f"q{oc}", name=f"q{oc}")
                for oc in range(NC_CHUNKS)
            ]
            k_sb = [
                persist.tile([128, S], bf16, tag=f"k{oc}", name=f"k{oc}")
                for oc in range(NC_CHUNKS)
            ]
            attn_sb = [
                persist.tile([128, S], bf16, tag=f"at{oc}", name=f"at{oc}")
                for oc in range(NC_CHUNKS)
            ]
            # per-chunk unnormalized attn rows 0:64 + denominator row 64
            AU = [
                persist.tile([D + 1, 2, S], f32, tag=f"au{c}", name=f"au{c}")
                for c in range(NC_CHUNKS)
            ]
            ot = persist.tile([128, NC_CHUNKS, S], bf16, tag="ot", name="ot")

            # ---- stage helpers ---------------------------------------
            def qkproj(oc):
                # Q then K chain; K bias dropped (softmax-invariant)
                ps_q = psA.tile([128, S], f32, tag="proj", name="ps_q")
                for cc in range(NC_CHUNKS):
                    nc.tensor.matmul(
                        ps_q[:], wslice(0, oc, cc), xt(cc),
                        start=(cc == 0), stop=(cc == NC_CHUNKS - 1),
                    )
                nc.vector.tensor_scalar_add(q_sb[oc][:], ps_q[:], cst[:, oc : oc + 1])
                ps_k = psA.tile([128, S], f32, tag="proj", name="ps_k")
                for cc in range(NC_CHUNKS):
                    nc.tensor.matmul(
                        ps_k[:], wslice(1, oc, cc), xt(cc),
                        start=(cc == 0), stop=(cc == NC_CHUNKS - 1),
                    )
                nc.vector.tensor_copy(k_sb[oc][:], ps_k[:])

            def scores_pair(oc, mid_hook=None):
                # Both heads of the chunk as row-tile pairs (partitions
                # 0:64 / 64:128 -> PE row groups 0-1 / 2-3): interleaved
                # emission lets the PE run them concurrently and pull
                # LDWEIGHTS ahead.  Causal trimming: psa = [kc0 q0:128 |
                # kc1 q0:256], psb = kc2 q0:384.  exp straight from
                # PSUM; 0/1 triangle applied after on the diagonal
                # sub-blocks (gpsimd, SBUF).
                Qh = [q_sb[oc][0:D, :], q_sb[oc][D : 2 * D, :]]
                Kh = [k_sb[oc][0:D, :], k_sb[oc][D : 2 * D, :]]
                psa = [
                    psS.tile([128, S], f32, tag="sc", name=f"psa{p}")
                    for p in range(2)
                ]
                for p in range(2):
                    nc.tensor.matmul(
                        psa[p][:, 0:128], Kh[p][:, 0:128], Qh[p][:, 0:128],
                        start=True, stop=True, skip_group_check=True,
                    )
                for p in range(2):
                    nc.tensor.matmul(
                        psa[p][:, 128:384], Kh[p][:, 128:256], Qh[p][:, 0:256],
                        start=True, stop=True, skip_group_check=True,
                    )
                psb = [
                    psS.tile([128, S], f32, tag="sc", name=f"psb{p}")
                    for p in range(2)
                ]
                for p in range(2):
                    nc.tensor.matmul(
                        psb[p][:], Kh[p][:, 256:384], Qh[p][:, 0:384],
                        start=True, stop=True, skip_group_check=True,
                    )
                out = []
                for p in range(2):
                    eA = epool.tile([128, 512], bf16, tag="eA", name="eA")
                    nc.scalar.activation(eA[:, 0:S], psa[p][:], Exp)
                    eB = epool.tile([128, S], bf16, tag="eB", name="eB")
                    nc.scalar.activation(eB[:], psb[p][:], Exp)
                    # eA is 512 wide so its two diagonal sub-blocks (cols
                    # 0:128 and 256:384) form one uniform-stride
                    # [128,2,128] AP for a single masked multiply.
                    diag2 = eA[:].rearrange("p (a q) -> p a q", q=256)[:, :, 0:128]
                    nc.gpsimd.tensor_mul(diag2, diag2, tri2)
                    nc.gpsimd.tensor_mul(eB[:, 256:384], eB[:, 256:384], tri2[:, 0, :])
                    out.append((eA, eB))
                    if p == 0 and mid_hook is not None:
                        mid_hook()
                return out

            def vproj():
                # lhsT = x block, rhs = wv; half 0 = heads 0-5 first so
                # chunk-0 attention can start after three chains
                for half in range(2):
                    for sq in range(3):
                        ps_v = psA.tile([128, S], f32, tag="proj", name="ps_v")
                        for cc in range(NC_CHUNKS):
                            nc.tensor.matmul(
                                ps_v[:],
                                xt(cc)[:, sq * 128 : (sq + 1) * 128],
                                wv_sb[:, half, cc, :],
                                start=(cc == 0), stop=(cc == NC_CHUNKS - 1),
                            )
                        dst = vt[sq][:, half * 6 : (half + 1) * 6, 0:D]
                        src = ps_v[:].rearrange("p (h d) -> p h d", d=D)
                        if half == 0:
                            nc.vector.tensor_copy(dst, src)
                        else:
                            nc.scalar.activation(dst, src, Copy)

            def av(h, eA, eB):
                # accumulate widest first so every element's first write
                # carries the start flag
                ps_av = psV.tile([D + 1, S], f32, tag="av", name="ps_av")
                nc.tensor.matmul(
                    ps_av[:, 0:384], vt[2][:, h, :], eB[:, 0:384],
                    start=True, stop=False, skip_group_check=True,
                )
                nc.tensor.matmul(
                    ps_av[:, 0:256], vt[1][:, h, :], eA[:, 128:384],
                    start=False, stop=False, skip_group_check=True,
                )
                nc.tensor.matmul(
                    ps_av[:, 0:128], vt[0][:, h, :], eA[:, 0:128],
                    start=False, stop=True, skip_group_check=True,
                )
                dst = AU[h // 2][0 : D + 1, h % 2, :]
                if h % 2 == 0:
                    nc.vector.tensor_copy(dst, ps_av[:, :])
                else:
                    nc.scalar.activation(dst, ps_av[:, :], Copy)

            def norm(c):
                # see docstring: gather -> reciprocal -> cast -> repack
                # -> K=2 bf16 broadcast matmul -> two multiplies
                s12 = npool.tile([12, 64], f32, tag="s12", name="s12")
                nc.sync.dma_start(s12[:], AU[c][D : D + 1, :, :])
                r12 = npool.tile([12, 64], f32, tag="r12", name="r12")
                nc.vector.reciprocal(r12[:], s12[:])
                rb12 = npool.tile([12, 64], bf16, tag="rb12", name="rb12")
                nc.scalar.activation(rb12[:], r12[:], Copy)
                rr = npool.tile([2, S], bf16, tag="rr", name="rr")
                nc.gpsimd.dma_start(
                    rr[:].rearrange("p (b q) -> p b q", q=64), rb12[:]
                )
                ps_n = psS.tile([128, S], f32, tag="sc", name="ps_n")
                nc.tensor.matmul(
                    ps_n[:], sel2[:], rr[:],
                    start=True, stop=True, skip_group_check=True,
                )
                for par in range(2):
                    nc.vector.tensor_mul(
                        attn_sb[c][par * D : (par + 1) * D, :],
                        AU[c][0:D, par, :],
                        ps_n[par * D : (par + 1) * D, :],
                    )

            o_ps = {}

            def oproj(oc, ccs, pool=None, stop=False):
                if oc in o_ps:
                    ps, start = o_ps[oc], False
                else:
                    tag = "proj" if pool is psA else ("av" if pool is psV else "sc")
                    ps, start = pool.tile(
                        [128, S], f32, tag=tag, name="ps_o"
                    ), True
                    o_ps[oc] = ps
                for i, cc in enumerate(ccs):
                    nc.tensor.matmul(
                        ps[:],
                        wo_sb[:, cc, oc * 128 : (oc + 1) * 128],
                        attn_sb[cc],
                        start=(start and i == 0),
                        stop=(stop and i == len(ccs) - 1),
                        skip_group_check=True,
                    )
                if stop:
                    del o_ps[oc]
                    nc.scalar.activation(
                        ot[:, oc, :], ps[:], Ident, bias=cst[:, 6 + oc : 7 + oc]
                    )
                    if oc % 2 == 1:
                        nc.sync.dma_start(
                            y_d[:, oc - 1 : oc + 1, :], ot[:, oc - 1 : oc + 1, :]
                        )

            # ---- schedule --------------------------------------------
            e_tiles = {}

            def run_pair(oc, mid_hook=None):
                pair = scores_pair(oc, mid_hook)
                e_tiles[2 * oc] = pair[0]
                e_tiles[2 * oc + 1] = pair[1]

            def run_av_norm(c):
                av(2 * c, *e_tiles.pop(2 * c))
                av(2 * c + 1, *e_tiles.pop(2 * c + 1))
                norm(c)

            qkproj(0)
            _phase2_loads()
            run_pair(0, mid_hook=_phase3_loads)
            for oc in (1, 2, 3, 4):
                qkproj(oc)
                run_pair(oc)
            vproj()
            run_av_norm(0)
            qkproj(5)
            run_pair(5)
            for c in (1, 2, 3):
                run_av_norm(c)
            oproj(0, (0, 1, 2, 3), pool=psA)
            oproj(1, (0, 1, 2, 3), pool=psA)
            run_av_norm(4)
            run_av_norm(5)
            oproj(2, (0, 1, 2, 3, 4), pool=psV)
            oproj(3, (0, 1, 2, 3, 4), pool=psV)
            oproj(0, (4,))
            oproj(1, (4,))
            oproj(4, (0, 1, 2, 3, 4), pool=psS)
            oproj(5, (0, 1, 2, 3, 4), pool=psS)
            for oc in range(NC_CHUNKS):
                oproj(oc, (5,), stop=True)

    return nc


def _get_nc():
    if "nc" not in _STATE:
        _STATE["nc"] = _build_nc()
    return _STATE["nc"]


# --------------------------------------------------------------------------
def _prep_maps(inputs):
    import ml_dtypes

    bf16 = ml_dtypes.bfloat16

    hs = np.asarray(inputs["hidden_states"], dtype=np.float32)
    Wq = np.asarray(inputs["Wq"], dtype=np.float32)
    bq = np.asarray(inputs["bq"], dtype=np.float32)
    Wk = np.asarray(inputs["Wk"], dtype=np.float32)
    Wv = np.asarray(inputs["Wv"], dtype=np.float32)
    bv = np.asarray(inputs["bv"], dtype=np.float32)
    Wo = np.asarray(inputs["Wo"], dtype=np.float32)

    # head-major channel permutation: c' = h*64 + d  <-  c = d*12 + h
    idx = (np.arange(H)[:, None] + np.arange(D)[None, :] * H).reshape(C)
    scale = float(D) ** -0.5

    wqt = np.ascontiguousarray((scale * Wq[idx, :]).T).astype(bf16)
    wkt = np.ascontiguousarray(Wk[idx, :].T).astype(bf16)
    wvt = np.ascontiguousarray(Wv[idx, :].T).astype(bf16)
    wot = np.ascontiguousarray(Wo.T).astype(bf16)

    # packed constants [128, 204] f32
    cstf = np.zeros((128, 204), dtype=np.float32)
    cstf[:, 0:6] = (scale * bq[idx]).reshape(NC_CHUNKS, 128).T
    # V-bias folded through attention (softmax rows sum to 1)
    cstf[:, 6:12] = (Wo @ bv[idx]).reshape(NC_CHUNKS, 128).T
    # 0/1 triangle: allowed keys are k >= q -> tri[k, q] = 1 iff k >= q
    tri = np.tril(np.ones((128, 128), dtype=np.float32)).astype(bf16)
    cstf[:, 12:140] = np.tile(tri, (1, 2)).view(np.float32)
    sel = np.zeros((2, 128), dtype=np.float32)
    sel[0, 0:64] = 1.0
    sel[1, 64:128] = 1.0
    cstf[0:2, 140:204] = sel.astype(bf16).view(np.float32)

    # pack [c_in, c_out] weights into their SBUF layouts (see _build_nc)
    nch = NC_CHUNKS
    wqp = np.ascontiguousarray(
        wqt.reshape(nch, 128, nch, 128).transpose(1, 2, 0, 3)
    )  # [p, out_chunk, in_chunk, col]
    wkp = np.ascontiguousarray(wkt.reshape(nch, 128, nch, 128).transpose(1, 2, 0, 3))
    wvp = np.ascontiguousarray(
        wvt.reshape(nch, 128, 2, 384).transpose(1, 2, 0, 3)
    )  # [p, half, in_chunk, col]
    wop = np.ascontiguousarray(wot.reshape(nch, 128, C).transpose(1, 0, 2))

    t2 = np.stack([wqp[:, 1:6], wkp[:, 1:6]], axis=2)  # [128, 5, 2, 6, 128]
    shared = {
        "t20": np.ascontiguousarray(t2[:, 0:1]),
        "t21": np.ascontiguousarray(t2[:, 1:2]),
        "t22": np.ascontiguousarray(t2[:, 2:3]),
        "t23": np.ascontiguousarray(t2[:, 3:5]),
        "wvt": wvp,
        "wot": wop,
        "cst": cstf,
    }
    maps = []
    for b in range(B):
        xb = hs[b, :, 0, :].astype(bf16)
        xp = xb.reshape(nch, 128, S).transpose(1, 0, 2).reshape(128, nch * S)
        t1 = np.ascontiguousarray(
            np.concatenate(
                [xp, wqp[:, 0].reshape(128, 768), wkp[:, 0].reshape(128, 768)],
                axis=1,
            )
        )
        maps.append({"t1": t1, **shared})
    return maps


def _run(inputs, trace=False, **kwargs):
    from concourse.bass_utils import run_bass_kernel_spmd

    nc = _get_nc()
    in_maps = _prep_maps(inputs)
    res = run_bass_kernel_spmd(
        nc, in_maps, core_ids=list(range(B)), trace=trace, **kwargs
    )
    out = np.stack(
        [
            np.asarray(res.results[b]["y"])
            .astype(np.float32)
            .transpose(1, 0, 2)  # [p, cc, s] -> [cc, p, s]
            .reshape(C, S)
            for b in range(B)
        ],
        axis=0,
    )
    return out.reshape(B, C, 1, S), res


def kernel(**inputs):
    out, _ = _run(inputs, trace=False)
    return out
